# revision 8
# baseline (speedup 1.0000x reference)
"""Trainium2 Bass kernel for nn_AdjustLayer (1x1 conv+BN -> 3x3 conv+BN+ReLU -> PrRoIPool).

Data-parallel over batch: 64 samples sharded 8-per-core across 8 NeuronCores.
Weights replicated. Returns (h, rf, roi5) like the reference.
"""

import os
import sys
import time

import numpy as np

sys.path.insert(0, "/opt/trn_rl_repo")

# ---- problem constants (hardcoded per contest contract) ----
B = 64
CIN = 1024
COUT = 256
H = W = 36
HW = H * W            # 1296
NCORES = 8
NPC = B // NCORES     # 8 samples per core
STRIDE = 8
POOL_TMP = 8
BN_EPS = 1e-5

PWID = 38             # padded row width for conv2 input layout
P0 = 1                # leading guard column in hp
HPL = P0 + PWID * PWID + 3   # 1448; conv2 reads [0, 1446)
NKB = (HW + 127) // 128      # 11 k-blocks for pooling contraction
YCHUNK = 12           # output rows per conv psum chunk (3 chunks of 12)

TRACE = False         # set by test.py to collect HW exec time
LAST = {}             # exec_time_ns etc. stashed here for test.py

_MODULE_CACHE = {}


def _ensure_ntff_hook():
    """Register the axon NTFF profile hook that concourse expects; the
    image's antenv package lacks axon_hooks, so provide it in-process."""
    import types

    try:
        from antenv.axon_hooks import get_axon_ntff_profile_hook
        if get_axon_ntff_profile_hook() is not None:
            return True
    except ImportError:
        pass
    try:
        import antenv
        from trn_agent_boot.trn_boot import _ntff_profile_via_ctypes
    except ImportError:
        return False
    hook = _ntff_profile_via_ctypes("/opt/axon/libaxon_pjrt.so")
    if hook is None:
        return False
    mod = types.ModuleType("antenv.axon_hooks")
    state = {"hook": hook}
    mod.set_axon_ntff_profile_hook = lambda h: state.__setitem__("hook", h)
    mod.get_axon_ntff_profile_hook = lambda: state["hook"]
    sys.modules["antenv.axon_hooks"] = mod
    antenv.axon_hooks = mod

    # avoid the S3 artifact upload in the trace path (no creds here)
    from concourse import bass_utils
    bass_utils.upload_artifacts = lambda tmpdir: tmpdir
    return True


def _build_module(ph, pw):
    import concourse.bass as bass  # noqa: F401
    import concourse.mybir as mybir
    import concourse.tile as tile
    from concourse import bacc

    dt = mybir.dt
    f32, f32r = dt.float32, dt.float32r
    AF = mybir.ActivationFunctionType
    npq = ph * pw

    nc = bacc.Bacc("TRN2", target_bir_lowering=False, debug=False)

    xs = nc.dram_tensor("xs", [NPC, CIN, HW], f32r, kind="ExternalInput")
    w1t = nc.dram_tensor("w1t", [8, 128, COUT], f32r, kind="ExternalInput")
    w2t = nc.dram_tensor("w2t", [2, 128, 9, COUT], f32r, kind="ExternalInput")
    bnb = nc.dram_tensor("bnb", [128, 4], f32, kind="ExternalInput")
    kmt = nc.dram_tensor("kmt", [NPC, 128, NKB, npq], f32r, kind="ExternalInput")
    idn = nc.dram_tensor("idn", [128, 128], f32r, kind="ExternalInput")
    hpz = nc.dram_tensor("hpz", [128, HPL], f32r, kind="ExternalInput")
    h_out = nc.dram_tensor("h_out", [NPC, COUT, HW], f32r, kind="ExternalOutput")
    rf_out = nc.dram_tensor("rf_out", [NPC, COUT, npq], f32, kind="ExternalOutput")

    with tile.TileContext(nc) as tc:
        with (
            tc.tile_pool(name="wpool", bufs=1) as wpool,
            tc.tile_pool(name="xpool", bufs=3) as xpool,
            tc.tile_pool(name="hppool", bufs=1) as hppool,
            tc.tile_pool(name="h2pool", bufs=2) as h2pool,
            tc.tile_pool(name="ftpool", bufs=2) as ftpool,
            tc.tile_pool(name="kmpool", bufs=2) as kmpool,
            tc.tile_pool(name="rfpool", bufs=2) as rfpool,
            tc.tile_pool(name="pmm", bufs=4, space="PSUM") as pmm,
            tc.tile_pool(name="ptr", bufs=2, space="PSUM") as ptr,
        ):
            # ---- replicated weights / constants ----
            w1s = wpool.tile([128, 8, COUT], f32r, tag="w1s")
            nc.sync.dma_start(w1s[:], w1t.ap().rearrange("k p m -> p k m"))
            w2s = []
            for kc in range(2):
                t = wpool.tile([128, 9, COUT], f32r, tag=f"w2s{kc}")
                nc.sync.dma_start(t[:], w2t[kc])
                w2s.append(t)
            bns = wpool.tile([128, 4], f32, tag="bns")
            nc.sync.dma_start(bns[:], bnb.ap())
            ident = wpool.tile([128, 128], f32r, tag="ident")
            nc.sync.dma_start(ident[:], idn.ap())

            # hp: persistent zero-padded conv2 input buffers; the padding is
            # zeroed once here and only the 36x36 interior is rewritten per
            # sample, so it stays zero.
            hp = []
            for kc in range(2):
                t = hppool.tile([128, HPL], f32r, tag=f"hp{kc}")
                nc.sync.dma_start(t[:], hpz.ap())
                hp.append(t)

            for n in range(NPC):
                # ---- load x (two halves of 512 input channels) ----
                xh = []
                for hf in range(2):
                    t = xpool.tile([128, 4, HW], f32r, tag="xh")
                    nc.sync.dma_start(
                        t[:],
                        xs[n, hf * 512:(hf + 1) * 512].rearrange(
                            "(a p) f -> p a f", p=128
                        ),
                    )
                    xh.append(t)

                # ---- load pooling kernel matrix early (overlaps compute) ----
                kms = kmpool.tile([128, NKB * npq], f32r, tag="km")
                nc.sync.dma_start(
                    kms[:], kmt[n].rearrange("p b q -> p (b q)")
                )

                # ---- conv1 (1x1) + BN1 -> hp padded layout ----
                for m in range(2):
                    for ch in range(3):
                        y0 = YCHUNK * ch
                        ncols = YCHUNK * W  # 432
                        ps = pmm.tile([128, 456], f32, tag="mm")
                        for k in range(8):
                            nc.tensor.matmul(
                                ps[:, 0:ncols],
                                w1s[:, k, m * 128:(m + 1) * 128],
                                xh[k // 4][:, k % 4, y0 * W:(y0 + YCHUNK) * W],
                                start=(k == 0),
                                stop=(k == 7),
                            )
                        dst = hp[m][
                            :, P0 + (y0 + 1) * PWID + 1:
                            P0 + (y0 + 1 + YCHUNK) * PWID + 1
                        ].rearrange("p (r c) -> p r c", c=PWID)[:, :, 0:W]
                        nc.scalar.activation(
                            dst,
                            ps[:, 0:ncols].rearrange("p (r c) -> p r c", c=W),
                            AF.Identity,
                            bias=bns[:, m:m + 1],
                            scale=1.0,
                        )

                # ---- conv2 (3x3) + BN2 + ReLU -> h2 dense ----
                h2ts = []
                for m in range(2):
                    h2t = h2pool.tile([128, HW], f32r, tag=f"h2{m}")
                    for ch in range(3):
                        y0 = YCHUNK * ch
                        ncols = YCHUNK * PWID  # 456
                        ps = pmm.tile([128, 456], f32, tag="mm")
                        first = True
                        for kc in range(2):
                            for tap in range(9):
                                dy, dx = tap // 3, tap % 3
                                base = (y0 + dy) * PWID + dx
                                nc.tensor.matmul(
                                    ps[:, 0:ncols],
                                    w2s[kc][:, tap, m * 128:(m + 1) * 128],
                                    hp[kc][:, base:base + ncols],
                                    start=first,
                                    stop=(kc == 1 and tap == 8),
                                )
                                first = False
                        src = ps[:, 0:ncols].rearrange(
                            "p (r c) -> p r c", c=PWID
                        )[:, :, 1:37]
                        nc.scalar.activation(
                            h2t[:, y0 * W:(y0 + YCHUNK) * W].rearrange(
                                "p (r c) -> p r c", c=W
                            ),
                            src,
                            AF.Relu,
                            bias=bns[:, 2 + m:3 + m],
                            scale=1.0,
                        )
                    nc.scalar.dma_start(
                        h_out[n, m * 128:(m + 1) * 128, :], h2t[:]
                    )
                    h2ts.append(h2t)

                # ---- transpose h2 -> featT [hw, c] via PE ----
                ftt = ftpool.tile([128, NKB * COUT], f32r, tag="ft")
                for b in range(NKB):
                    wcols = min(128, HW - 128 * b)
                    for m in range(2):
                        tp = ptr.tile([128, 128], f32r, tag="tp")
                        nc.tensor.transpose(
                            tp[0:wcols, 0:128],
                            h2ts[m][:, 128 * b:128 * b + wcols],
                            ident[:],
                        )
                        nc.vector.tensor_copy(
                            ftt[0:wcols, b * COUT + m * 128:
                                b * COUT + (m + 1) * 128],
                            tp[0:wcols, 0:128],
                        )

                # ---- pooling: rf[c, pq] = sum_hw featT[hw, c] * km[hw, pq] ----
                for m in range(2):
                    rp = pmm.tile([128, npq], f32, tag="mm")
                    for b in range(NKB):
                        kb = min(128, HW - 128 * b)
                        nc.tensor.matmul(
                            rp[:],
                            ftt[0:kb, b * COUT + m * 128:b * COUT + (m + 1) * 128],
                            kms[0:kb, b * npq:(b + 1) * npq],
                            start=(b == 0),
                            stop=(b == NKB - 1),
                        )
                    rfs = rfpool.tile([128, npq], f32, tag="rf")
                    nc.vector.tensor_copy(rfs[:], rp[:])
                    nc.scalar.dma_start(
                        rf_out[n, m * 128:(m + 1) * 128, :], rfs[:]
                    )

    nc.compile()
    return nc


def _get_module(ph, pw):
    key = (ph, pw)
    if key not in _MODULE_CACHE:
        _MODULE_CACHE[key] = _build_module(ph, pw)
    return _MODULE_CACHE[key]


def _hat_int(a, b, g):
    # exact integral over [a, b] of max(0, 1 - |t - g|); a,b: [N,P]; g: [G]
    def F(s):
        s = np.clip(s, -1.0, 1.0)
        return s - np.sign(s) * 0.5 * s * s

    return F(b[:, :, None] - g) - F(a[:, :, None] - g)


def _host_prep(x, roi, w1, g1, b1, m1, v1, w2, g2, b2, m2, v2, ph, pw):
    f32 = np.float32
    npq = ph * pw

    s1 = (g1 / np.sqrt(v1 + BN_EPS)).astype(f32)
    t1 = (b1 - m1 * s1).astype(f32)
    s2 = (g2 / np.sqrt(v2 + BN_EPS)).astype(f32)
    t2 = (b2 - m2 * s2).astype(f32)

    w1_eff = (w1[:, :, 0, 0] * s1[:, None]).astype(f32)          # [COUT, CIN]
    w1t = np.ascontiguousarray(w1_eff.T.reshape(8, 128, COUT))   # [k, p, m]
    w2_eff = (w2 * s2[:, None, None, None]).astype(f32)          # [COUT,COUT,3,3]
    w2t = np.ascontiguousarray(
        w2_eff.transpose(1, 2, 3, 0).reshape(2, 128, 9, COUT)
    )
    bnb = np.stack(
        [t1[:128], t1[128:], t2[:128], t2[128:]], axis=1
    ).astype(f32)                                                # [128, 4]

    # pooling kernel matrices from roi (xywh -> xyxy, feature scale 1/STRIDE)
    scale = f32(1.0 / STRIDE)
    x1 = (roi[:, 0] * scale).astype(f32)
    y1 = (roi[:, 1] * scale).astype(f32)
    x2 = ((roi[:, 0] + roi[:, 2]) * scale).astype(f32)
    y2 = ((roi[:, 1] + roi[:, 3]) * scale).astype(f32)
    bw = ((x2 - x1) / pw).astype(f32)
    bh = ((y2 - y1) / ph).astype(f32)
    ya = y1[:, None] + bh[:, None] * np.arange(ph, dtype=f32)
    xa = x1[:, None] + bw[:, None] * np.arange(pw, dtype=f32)
    gh = np.arange(H, dtype=f32)
    gw = np.arange(W, dtype=f32)
    wy = _hat_int(ya, ya + bh[:, None], gh).astype(f32)          # [B, ph, H]
    wx = _hat_int(xa, xa + bw[:, None], gw).astype(f32)          # [B, pw, W]
    area = np.maximum(bw * bh, 0.0)
    coef = np.where(area > 0, 1.0 / np.maximum(area, 1e-12), 0.0).astype(f32)

    km = np.einsum("nph,nqw->nhwpq", wy, wx).reshape(B, HW, npq)
    km *= coef[:, None, None]
    km_pad = np.zeros((B, NKB * 128, npq), f32)
    km_pad[:, :HW] = km
    kmt = np.ascontiguousarray(
        km_pad.reshape(B, NKB, 128, npq).transpose(0, 2, 1, 3)
    )                                                            # [B,128,NKB,npq]

    bidx = np.arange(B, dtype=f32).reshape(-1, 1)
    roi5 = np.concatenate(
        [bidx, roi[:, 0:2], roi[:, 0:2] + roi[:, 2:4]], axis=-1
    ).astype(f32)

    return w1t, w2t, bnb, kmt, roi5


def kernel(x, roi, w1, g1, b1, m1, v1, w2, g2, b2, m2, v2,
           tmp_flag, pooled_height_src, pooled_width_src):
    from concourse.bass_utils import run_bass_kernel_spmd

    x = np.ascontiguousarray(np.asarray(x, dtype=np.float32))
    roi = np.asarray(roi, dtype=np.float32)
    if int(np.asarray(tmp_flag)):
        ph = pw = POOL_TMP
    else:
        ph = int(np.asarray(pooled_height_src))
        pw = int(np.asarray(pooled_width_src))
    npq = ph * pw

    w1t, w2t, bnb, kmt, roi5 = _host_prep(
        np.asarray(x), roi,
        *[np.asarray(a, dtype=np.float32) for a in
          (w1, g1, b1, m1, v1, w2, g2, b2, m2, v2)],
        ph, pw,
    )

    nc = _get_module(ph, pw)

    xr = x.reshape(B, CIN, HW)
    idn = np.eye(128, dtype=np.float32)
    hpz = np.zeros((128, HPL), dtype=np.float32)
    in_maps = []
    for c in range(NCORES):
        sl = slice(c * NPC, (c + 1) * NPC)
        in_maps.append({
            "xs": np.ascontiguousarray(xr[sl]),
            "w1t": w1t,
            "w2t": w2t,
            "bnb": bnb,
            "kmt": np.ascontiguousarray(kmt[sl]),
            "idn": idn,
            "hpz": hpz,
        })

    trace = TRACE and _ensure_ntff_hook()
    tmpdir = None
    if trace:
        tmpdir = os.path.join("/root/problem/traces", time.strftime("%H%M%S"))
        os.makedirs(tmpdir, exist_ok=True)
        LAST["trace_dir"] = tmpdir
    t0 = time.monotonic()
    res = run_bass_kernel_spmd(
        nc, in_maps, list(range(NCORES)), trace=trace, tmpdir=tmpdir
    )
    LAST["wall_s"] = time.monotonic() - t0
    LAST["exec_time_ns"] = res.exec_time_ns
    LAST["mean_exec_time_ns"] = res.mean_exec_time_ns
    LAST["results"] = res

    h = np.concatenate(
        [r["h_out"] for r in res.results], axis=0
    ).reshape(B, COUT, H, W).astype(np.float32)
    rf = np.concatenate(
        [r["rf_out"] for r in res.results], axis=0
    ).reshape(B, COUT, ph, pw).astype(np.float32)
    return h, rf, roi5
